# revision 18
# baseline (speedup 1.0000x reference)
"""GCN link predictor on 8 trn2 NeuronCores (Bass/Tile SPMD).

Key design (vs the original per-tile indirect_dma_start baseline, which spent
~1 us of Pool-engine SWDGE time per 128-row gather, ~11.5k times):

  - All gathers go through batched `dma_gather` (int16 indices, 25%-of-N
    chunks so indices fit int16; one call per ~30-32 tiles) -> ~100x fewer
    Pool-engine descriptor-generation ops.
  - All gather tables are fp16 with 128 columns (256B rows, the dma_gather
    minimum element size). Layer 3 (COUT=64) is zero-padded to 128 cols via
    a padded W3/b3, so layer-3 aggregation and the decode dot products run
    through identical 128-col code and the pad contributes exactly 0.
  - Aggregation: edges (+self-loops) partitioned by dst owner, grouped by
    (src-chunk pass q, dst block b), padded to a uniform K_q tiles per
    block; S matrices (is_equal vs iota) built 4 tiles per DVE op; PE
    matmuls accumulate each (q,b) segment in PSUM, then DVE adds into an
    SBUF f32 accumulator; epilogue scales by dinv[dst], adds bias (+relu)
    into the fp16 SBUF-resident z slice.
  - Decode: edges sorted by (src chunk, dst chunk) into 16 groups padded to
    a uniform TG tiles; per 32-tile batch two dma_gathers [P, 32, 128] fp16,
    DVE mult + 3D reduce -> score columns; PE-transpose -> scores in sorted
    order; the host inverse-permutes.

Self-contained: hardcodes all shapes for the nn_GCNLinkPredictor problem.
"""
import numpy as np

import concourse.bacc as bacc
import concourse.bass as bass
import concourse.mybir as mybir
import concourse.tile as tile
from concourse.bass_utils import run_bass_kernel_spmd
from concourse.masks import make_identity

P = 128
N = 100000
E = 1600000
M = 8
NQ = 4                       # src chunks (int16 index ranges)
NPC = N // M                 # 12500
BPC = (NPC + P - 1) // P     # 98
SLICE = BPC * P              # 12544
CHN = N // NQ                # 25000 nodes per chunk
CIN = 128
CH = 128                     # unified on-device column count (incl layer 3)
COUT = 64
EPC = E // M                 # 200000

SB = 4         # S-build batch (tiles per DVE op)
GBT = 32       # target gather batch (tiles per dma_gather call)
BDD = 32       # decode tiles per gather batch

PHASES = None  # None = all; else set of phase names to include


def _configure(n, e):
    """Switch module to a smaller problem size (for fast validation)."""
    global N, E, NPC, BPC, SLICE, EPC, CHN
    N, E = n, e
    NPC = N // M
    BPC = (NPC + P - 1) // P
    SLICE = BPC * P
    EPC = E // M
    CHN = N // NQ

f32 = mybir.dt.float32
f16 = mybir.dt.float16
i16 = mybir.dt.int16
i32 = mybir.dt.int32


def _wrap16(idx):
    """[n] int array -> dma_gather idx table [128, n//16] int16
    (idx i at [i%16, i//16], replicated to all 8 16-partition groups)."""
    n = idx.size
    a = np.ascontiguousarray(idx.reshape(n // 16, 16).T.astype(np.int16))
    return np.ascontiguousarray(np.tile(a, (8, 1)))


# --------------------------- host preprocessing ---------------------------

def _prep_agg(pos_edge_index):
    # deg/dinv INCLUDE self-loops; the edge stream EXCLUDES them (the
    # self-loop term is added in the aggregation epilogue from hhat).
    src = pos_edge_index[0]
    dst = pos_edge_index[1]
    deg = (np.bincount(dst, minlength=N) + 1).astype(np.float32)
    dinv = (1.0 / np.sqrt(deg)).astype(np.float32)

    q_all = src // CHN
    order = np.lexsort((q_all, dst))
    src_s, dst_s, q_s = src[order], dst[order], q_all[order]
    core_of = dst_s // NPC

    # per-(core, block, q) counts -> uniform K_q
    blk_all = (dst_s - core_of * NPC) // P
    counts = np.zeros((M, BPC, NQ), dtype=np.int64)
    np.add.at(counts, (core_of, blk_all, q_s), 1)
    Kq = [int(np.max((counts[:, :, q] + P - 1) // P)) for q in range(NQ)]
    Tt = BPC * int(np.sum(Kq))
    qbase = np.concatenate([[0], np.cumsum(Kq)]) * BPC  # tile base per pass

    cores = []
    for c in range(M):
        sel = core_of == c
        s_c, d_c, q_c = src_s[sel], dst_s[sel], q_s[sel]
        b_c = (d_c - c * NPC) // P
        key = b_c * NQ + q_c
        ord2 = np.argsort(key, kind="stable")
        s_c, d_c, key = s_c[ord2], d_c[ord2], key[ord2]
        srcL = np.zeros(Tt * P, dtype=np.int64)
        dstloc = np.full(Tt * P, -1.0, dtype=np.float32)
        bounds = np.searchsorted(key, np.arange(BPC * NQ + 1))
        for b in range(BPC):
            for q in range(NQ):
                lo, hi = bounds[b * NQ + q], bounds[b * NQ + q + 1]
                n_e = hi - lo
                base = (qbase[q] + b * Kq[q]) * P
                srcL[base:base + n_e] = s_c[lo:hi] - q * CHN
                dstloc[base:base + n_e] = (d_c[lo:hi] -
                                           (c * NPC + b * P)).astype(np.float32)
        dinvT = np.zeros((P, BPC), dtype=np.float32)
        for b in range(BPC):
            lo = c * NPC + b * P
            hi = min(lo + P, (c + 1) * NPC)
            dinvT[:hi - lo, b] = dinv[lo:hi]
        cores.append(dict(
            srcT16=_wrap16(srcL),
            dstlocT=np.ascontiguousarray(
                dstloc.reshape(Tt, P).T.astype(np.float16)),
            dinvT=dinvT))
    return Kq, Tt, cores


def _prep_decode(pe, ne):
    """Sort each core's edges by (src chunk, dst chunk); uniform TG tiles
    per group. Returns (TG, per-set per-core dict with idx tables + pos)."""
    sets = [pe, ne]
    keys, counts = [], np.zeros((2, M, NQ * NQ), np.int64)
    for si, ei in enumerate(sets):
        kk = []
        for c in range(M):
            s = ei[0, c * EPC:(c + 1) * EPC]
            d = ei[1, c * EPC:(c + 1) * EPC]
            key = (s // CHN) * NQ + (d // CHN)
            np.add.at(counts[si, c], key, 1)
            kk.append(key)
        keys.append(kk)
    TG = int(np.max((counts + P - 1) // P))
    NT = NQ * NQ * TG
    out = []
    for si, ei in enumerate(sets):
        cores = []
        for c in range(M):
            s = ei[0, c * EPC:(c + 1) * EPC].astype(np.int64)
            d = ei[1, c * EPC:(c + 1) * EPC].astype(np.int64)
            key = keys[si][c]
            order = np.argsort(key, kind="stable")
            s_o, d_o, key_o = s[order], d[order], key[order]
            sL = np.zeros(NT * P, dtype=np.int64)
            dL = np.zeros(NT * P, dtype=np.int64)
            pos = np.full(NT * P, -1, dtype=np.int64)
            bounds = np.searchsorted(key_o, np.arange(NQ * NQ + 1))
            for g in range(NQ * NQ):
                lo, hi = bounds[g], bounds[g + 1]
                n_e = hi - lo
                base = g * TG * P
                sL[base:base + n_e] = s_o[lo:hi] - (g // NQ) * CHN
                dL[base:base + n_e] = d_o[lo:hi] - (g % NQ) * CHN
                pos[base:base + n_e] = order[lo:hi]
            cores.append(dict(sidxT=_wrap16(sL), didxT=_wrap16(dL), pos=pos))
        out.append(cores)
    return TG, out


# ----------------------------- device builder -----------------------------

def build_nc(Kq, TG, reps=1):
    Tt = BPC * int(np.sum(Kq))
    qbase = np.concatenate([[0], np.cumsum(Kq)]) * BPC
    NT = NQ * NQ * TG
    DGd = (NT + P - 1) // P          # decode transpose groups
    NTP = DGd * P
    nc = bacc.Bacc(None, target_bir_lowering=False)
    with tile.TileContext(nc) as tc:
        with tc.tile_pool(name="dram", bufs=1, space="DRAM") as dram, \
             tc.tile_pool(name="cst", bufs=1) as cst, \
             tc.tile_pool(name="xt", bufs=4) as xtp, \
             tc.tile_pool(name="sS", bufs=4) as sSp, \
             tc.tile_pool(name="msg", bufs=2) as msgp, \
             tc.tile_pool(name="prod", bufs=2) as prodp, \
             tc.tile_pool(name="zb", bufs=4) as zbp, \
             tc.tile_pool(name="ps", bufs=2, space="PSUM") as psp, \
             tc.tile_pool(name="acc", bufs=2, space="PSUM") as accp:

            # ---------------- I/O ----------------
            def ein(name, shape, dtype=f32):
                return dram.tile(shape, dtype, kind="ExternalInput",
                                 name=name, uniquify=False)

            x_s = ein("x_s", [SLICE, CIN], f16)
            W1 = ein("W1", [CIN, CH], f16); W2 = ein("W2", [CH, CH], f16)
            W3 = ein("W3", [CH, CH], f16)          # host-padded to 128 cols
            bb1 = ein("bb1", [P, CH]); bb2 = ein("bb2", [P, CH])
            bb3 = ein("bb3", [P, CH])              # host-padded
            dinvT = ein("dinvT", [P, BPC])
            srcT16 = ein("srcT16", [P, Tt * 8], i16)
            dstlocT = ein("dstlocT", [P, Tt], f16)
            ps_idx = ein("ps_idx", [P, NT * 8], i16)
            pd_idx = ein("pd_idx", [P, NT * 8], i16)
            ns_idx = ein("ns_idx", [P, NT * 8], i16)
            nd_idx = ein("nd_idx", [P, NT * 8], i16)

            pos_out = dram.tile([NTP, P], f32, kind="ExternalOutput",
                                name="pos_out", uniquify=False)
            neg_out = dram.tile([NTP, P], f32, kind="ExternalOutput",
                                name="neg_out", uniquify=False)

            # internal DRAM
            hs1 = dram.tile([SLICE, CH], f16, name="hs1")
            hs2 = dram.tile([SLICE, CH], f16, name="hs2")
            hs3 = dram.tile([SLICE, CH], f16, name="hs3")
            z3s = dram.tile([SLICE, CH], f16, name="z3s")

            # ---------------- constants to SBUF ----------------
            W1_sb = cst.tile([CIN, CH], f16)
            W2_sb = cst.tile([CH, CH], f16)
            W3_sb = cst.tile([CH, CH], f16)
            bb1_sb = cst.tile([P, CH], f32)
            bb2_sb = cst.tile([P, CH], f32)
            bb3_sb = cst.tile([P, CH], f32)
            dinv_sb = cst.tile([P, BPC], f32)
            dstloc_sb = cst.tile([P, Tt], f16)
            for dst_t, src_t in [(W1_sb, W1), (W2_sb, W2), (W3_sb, W3),
                                 (bb1_sb, bb1), (bb2_sb, bb2), (bb3_sb, bb3),
                                 (dinv_sb, dinvT), (dstloc_sb, dstlocT)]:
                nc.sync.dma_start(out=dst_t[:], in_=src_t[:])

            ident = cst.tile([P, P], f32)
            make_identity(nc, ident[:])
            ident_h = cst.tile([P, P], f16)
            nc.vector.tensor_copy(out=ident_h[:], in_=ident[:])
            iota_i = cst.tile([P, P], i32)
            nc.gpsimd.iota(iota_i[:], pattern=[[1, P]], base=0,
                           channel_multiplier=0)
            iota_f = cst.tile([P, P], f16)
            nc.vector.tensor_copy(out=iota_f[:], in_=iota_i[:])

            # persistent SBUF state
            z_sb = cst.tile([P, BPC * CH], f16)        # z1 / z2 slice
            acc_sb = cst.tile([P, BPC * CH], f32)      # agg accumulator
            hh_sb = cst.tile([P, BPC * CH], f16)       # own hhat slice
            score_sb = cst.tile([P, NTP], f32)
            nc.vector.memset(score_sb[:], 0.0)

            # ---------------- phases ----------------
            def dense(layer, W_sb, hs_out, scope):
                with nc.named_scope(scope):
                    for b in range(BPC):
                        if layer == 1:
                            zt = xtp.tile([P, CIN], f16, tag="zt")
                            nc.sync.dma_start(out=zt[:],
                                              in_=x_s[b * P:(b + 1) * P, :])
                            src_ap = zt[:]
                        else:
                            src_ap = z_sb[:, b * CH:(b + 1) * CH]
                        tp = psp.tile([P, CH], f16, tag="tp")
                        nc.tensor.transpose(out=tp[:], in_=src_ap,
                                            identity=ident_h[:])
                        zT = xtp.tile([P, CH], f16, tag="zT")
                        nc.vector.tensor_copy(out=zT[:], in_=tp[:])
                        hp = psp.tile([P, CH], f32, tag="hp")
                        nc.tensor.matmul(out=hp[:], lhsT=zT[:], rhs=W_sb[:],
                                         start=True, stop=True)
                        hh = hh_sb[:, b * CH:(b + 1) * CH]
                        nc.vector.tensor_scalar(
                            out=hh, in0=hp[:],
                            scalar1=dinv_sb[:, b:b + 1], scalar2=None,
                            op0=mybir.AluOpType.mult)
                        nc.sync.dma_start(
                            out=hs_out[b * P:(b + 1) * P, :], in_=hh)

            def allgather(slice_t, full_t, scope):
                with nc.named_scope(scope):
                    nc.gpsimd.collective_compute(
                        "AllGather", mybir.AluOpType.bypass,
                        replica_groups=[list(range(M))],
                        ins=[slice_t[:NPC, :]],
                        outs=[full_t[:]])

            def agg(hf, bias_sb, relu, scope):
                with nc.named_scope(scope):
                    for q in range(NQ):
                        KQ = Kq[q]
                        gbb = max(1, GBT // KQ)      # blocks per gather call
                        for b0 in range(0, BPC, gbb):
                            nblk = min(gbb, BPC - b0)
                            ntile = nblk * KQ
                            t0 = int(qbase[q]) + b0 * KQ
                            aidx = xtp.tile([P, gbb * KQ * 8], i16,
                                            tag="aidx")
                            nc.sync.dma_start(
                                out=aidx[:, :ntile * 8],
                                in_=srcT16[:, t0 * 8:(t0 + ntile) * 8])
                            msg = msgp.tile([P, gbb * KQ, CH], f16, tag="msg")
                            nc.gpsimd.dma_gather(
                                msg[:, :ntile, :],
                                hf[q * CHN:(q + 1) * CHN, :],
                                aidx[:, :ntile * 8],
                                ntile * P, ntile * P, CH,
                                single_packet=False)
                            for bi in range(nblk):
                                b = b0 + bi
                                seg = accp.tile([P, CH], f32, tag="seg")
                                for k0 in range(0, KQ, SB):
                                    nb = min(SB, KQ - k0)
                                    S4 = sSp.tile([P, SB, P], f16, tag="S4")
                                    tt = t0 + bi * KQ + k0
                                    nc.vector.tensor_tensor(
                                        out=S4[:, :nb, :],
                                        in0=dstloc_sb[:, tt:tt + nb]
                                            .unsqueeze(2)
                                            .to_broadcast([P, nb, P]),
                                        in1=iota_f[:].unsqueeze(1)
                                            .to_broadcast([P, nb, P]),
                                        op=mybir.AluOpType.is_equal)
                                    for j in range(nb):
                                        k = k0 + j
                                        nc.tensor.matmul(
                                            out=seg[:], lhsT=S4[:, j, :],
                                            rhs=msg[:, bi * KQ + k, :],
                                            start=(k == 0),
                                            stop=(k == KQ - 1))
                                a_sl = acc_sb[:, b * CH:(b + 1) * CH]
                                if q == 0:
                                    nc.vector.tensor_copy(out=a_sl, in_=seg[:])
                                else:
                                    nc.vector.tensor_tensor(
                                        out=a_sl, in0=a_sl, in1=seg[:],
                                        op=mybir.AluOpType.add)
                    for b in range(BPC):
                        # self-loop: z = (acc + hhat_own)*dinv + bias
                        t1 = zbp.tile([P, CH], f32, tag="t1")
                        nc.vector.tensor_scalar(
                            out=t1[:], in0=hh_sb[:, b * CH:(b + 1) * CH],
                            scalar1=dinv_sb[:, b:b + 1], scalar2=None,
                            op0=mybir.AluOpType.mult)
                        zb = zbp.tile([P, CH], f32, tag="zbE")
                        nc.vector.tensor_scalar(
                            out=zb[:], in0=acc_sb[:, b * CH:(b + 1) * CH],
                            scalar1=dinv_sb[:, b:b + 1], scalar2=None,
                            op0=mybir.AluOpType.mult)
                        nc.vector.tensor_tensor(
                            out=zb[:], in0=zb[:], in1=t1[:],
                            op=mybir.AluOpType.add)
                        if relu:
                            nc.vector.tensor_tensor(
                                out=zb[:], in0=zb[:], in1=bias_sb[:],
                                op=mybir.AluOpType.add)
                            nc.vector.tensor_scalar_max(
                                z_sb[:, b * CH:(b + 1) * CH], zb[:], 0.0)
                        else:
                            zb3 = zbp.tile([P, CH], f16, tag="zb3")
                            nc.vector.tensor_tensor(
                                out=zb3[:], in0=zb[:], in1=bias_sb[:],
                                op=mybir.AluOpType.add)
                            nc.sync.dma_start(
                                out=z3s[b * P:(b + 1) * P, :], in_=zb3[:])

            def decode(z3f, sidx_d, didx_d, out_t, scope):
                with nc.named_scope(scope):
                    for g in range(NQ * NQ):
                        qs, qd = g // NQ, g % NQ
                        for off in range(0, TG, BDD):
                            nt = min(BDD, TG - off)
                            t0 = g * TG + off
                            sidx = xtp.tile([P, BDD * 8], i16, tag="sidx")
                            didx = xtp.tile([P, BDD * 8], i16, tag="didx")
                            nc.sync.dma_start(
                                out=sidx[:, :nt * 8],
                                in_=sidx_d[:, t0 * 8:(t0 + nt) * 8])
                            nc.sync.dma_start(
                                out=didx[:, :nt * 8],
                                in_=didx_d[:, t0 * 8:(t0 + nt) * 8])
                            za = msgp.tile([P, BDD, CH], f16, tag="za")
                            zbt = msgp.tile([P, BDD, CH], f16, tag="zbt")
                            nc.gpsimd.dma_gather(
                                za[:, :nt, :],
                                z3f[qs * CHN:(qs + 1) * CHN, :],
                                sidx[:, :nt * 8], nt * P, nt * P, CH,
                                single_packet=False)
                            nc.gpsimd.dma_gather(
                                zbt[:, :nt, :],
                                z3f[qd * CHN:(qd + 1) * CHN, :],
                                didx[:, :nt * 8], nt * P, nt * P, CH,
                                single_packet=False)
                            prod = prodp.tile([P, BDD, CH], f16, tag="prod")
                            nc.vector.tensor_tensor(
                                out=prod[:, :nt, :], in0=za[:, :nt, :],
                                in1=zbt[:, :nt, :], op=mybir.AluOpType.mult)
                            nc.vector.tensor_reduce(
                                out=score_sb[:, t0:t0 + nt].unsqueeze(2),
                                in_=prod[:, :nt, :],
                                axis=mybir.AxisListType.X,
                                op=mybir.AluOpType.add)
                    for gg in range(DGd):
                        tp = psp.tile([P, P], f32, tag="tpS")
                        nc.tensor.transpose(
                            out=tp[:], in_=score_sb[:, gg * P:(gg + 1) * P],
                            identity=ident[:])
                        so = zbp.tile([P, P], f32, tag="so")
                        nc.vector.tensor_copy(out=so[:], in_=tp[:])
                        nc.sync.dma_start(
                            out=out_t[gg * P:(gg + 1) * P, :], in_=so[:])

            def on(p):
                return PHASES is None or p in PHASES

            def run_pipeline(r):
                # Shared (collective-output) DRAM tensors allow only a
                # single writer instruction -> one set per rep.
                hf1 = dram.tile([N, CH], f16, name=f"hf1_r{r}",
                                addr_space="Shared")
                hf2 = dram.tile([N, CH], f16, name=f"hf2_r{r}",
                                addr_space="Shared")
                hf3 = dram.tile([N, CH], f16, name=f"hf3_r{r}",
                                addr_space="Shared")
                z3f = dram.tile([N, CH], f16, name=f"z3f_r{r}",
                                addr_space="Shared")
                if on("dense1"):
                    dense(1, W1_sb, hs1, "dense1")
                if on("ag1"):
                    allgather(hs1, hf1, "ag1")
                if on("agg1"):
                    agg(hf1, bb1_sb, True, "agg1")
                if on("dense2"):
                    dense(2, W2_sb, hs2, "dense2")
                if on("ag2"):
                    allgather(hs2, hf2, "ag2")
                if on("agg2"):
                    agg(hf2, bb2_sb, True, "agg2")
                if on("dense3"):
                    dense(3, W3_sb, hs3, "dense3")
                if on("ag3"):
                    allgather(hs3, hf3, "ag3")
                if on("agg3"):
                    agg(hf3, bb3_sb, False, "agg3")
                if on("ag4"):
                    allgather(z3s, z3f, "ag4")
                if on("dec"):
                    decode(z3f, ps_idx, pd_idx, pos_out, "dec_pos")
                    decode(z3f, ns_idx, nd_idx, neg_out, "dec_neg")

            for r in range(reps):
                run_pipeline(r)
    nc.compile()
    return nc


_CACHE = {}


def _make_in_maps(x, W1, b1, W2, b2, W3, b3, pe, ne):
    Kq, Tt, agg_cores = _prep_agg(pe)
    TG, dec = _prep_decode(pe, ne)
    W3p = np.zeros((CH, CH), np.float16)
    W3p[:, :COUT] = W3.astype(np.float16)
    b3p = np.zeros(CH, np.float32)
    b3p[:COUT] = b3
    in_maps = []
    pos_maps = []
    for c in range(M):
        xs = np.zeros((SLICE, CIN), np.float16)
        xs[:NPC] = x[c * NPC:(c + 1) * NPC].astype(np.float16)
        a = agg_cores[c]
        in_maps.append({
            "x_s": xs,
            "W1": W1.astype(np.float16), "W2": W2.astype(np.float16),
            "W3": W3p,
            "bb1": np.tile(b1[None, :], (P, 1)).astype(np.float32),
            "bb2": np.tile(b2[None, :], (P, 1)).astype(np.float32),
            "bb3": np.tile(b3p[None, :], (P, 1)).astype(np.float32),
            "dinvT": a["dinvT"], "srcT16": a["srcT16"],
            "dstlocT": a["dstlocT"],
            "ps_idx": dec[0][c]["sidxT"], "pd_idx": dec[0][c]["didxT"],
            "ns_idx": dec[1][c]["sidxT"], "nd_idx": dec[1][c]["didxT"],
        })
        pos_maps.append((dec[0][c]["pos"], dec[1][c]["pos"]))
    return (Kq, TG), (in_maps, pos_maps)


def _run(run_args, key_args, reps=1):
    in_maps, pos_maps = run_args
    Kq, TG = key_args
    key = (tuple(Kq), TG, reps)
    if key not in _CACHE:
        _CACHE[key] = build_nc(list(Kq), TG, reps=reps)
    res = run_bass_kernel_spmd(_CACHE[key], in_maps,
                               core_ids=list(range(M)))
    pos = np.empty(E, np.float32)
    neg = np.empty(E, np.float32)
    for c in range(M):
        for name, pos_arr, out in [("pos_out", pos_maps[c][0], pos),
                                   ("neg_out", pos_maps[c][1], neg)]:
            flat = res.results[c][name].ravel()
            valid = pos_arr >= 0
            out[c * EPC:(c + 1) * EPC][pos_arr[valid]] = \
                flat[:pos_arr.size][valid]
    return pos, neg


def kernel(x, W1, b1, W2, b2, W3, b3, pos_edge_index, neg_edge_index):
    x = np.asarray(x, dtype=np.float32)
    W1 = np.asarray(W1, np.float32); b1 = np.asarray(b1, np.float32)
    W2 = np.asarray(W2, np.float32); b2 = np.asarray(b2, np.float32)
    W3 = np.asarray(W3, np.float32); b3 = np.asarray(b3, np.float32)
    pe = np.asarray(pos_edge_index).astype(np.int64)
    ne = np.asarray(neg_edge_index).astype(np.int64)
    key_args, run_args = _make_in_maps(x, W1, b1, W2, b2, W3, b3, pe, ne)
    return _run(run_args, key_args, reps=1)


# revision 35
# speedup vs baseline: 1.4984x; 1.4984x over previous
"""GCN link predictor on 8 trn2 NeuronCores (Bass/Tile SPMD).

Key design (vs the original per-tile indirect_dma_start baseline, which spent
~1 us of Pool-engine SWDGE time per 128-row gather, ~11.5k times):

  - All gathers go through batched `dma_gather` (int16 indices, 25%-of-N
    chunks so indices fit int16; one call per ~30-32 tiles) -> ~100x fewer
    Pool-engine descriptor-generation ops.
  - All gather tables are fp16 with 128 columns (256B rows, the dma_gather
    minimum element size). Layer 3 (COUT=64) is zero-padded to 128 cols via
    a padded W3/b3, so layer-3 aggregation and the decode dot products run
    through identical 128-col code and the pad contributes exactly 0.
  - Aggregation: edges (+self-loops) partitioned by dst owner, grouped by
    (src-chunk pass q, dst block b), padded to a uniform K_q tiles per
    block; S matrices (is_equal vs iota) built 4 tiles per DVE op; PE
    matmuls accumulate each (q,b) segment in PSUM, then DVE adds into an
    SBUF f32 accumulator; epilogue scales by dinv[dst], adds bias (+relu)
    into the fp16 SBUF-resident z slice.
  - Decode: edges sorted by (src chunk, dst chunk) into 16 groups padded to
    a uniform TG tiles; per 32-tile batch two dma_gathers [P, 32, 128] fp16,
    DVE mult + 3D reduce -> score columns; PE-transpose -> scores in sorted
    order; the host inverse-permutes.

Self-contained: hardcodes all shapes for the nn_GCNLinkPredictor problem.
"""
import numpy as np

import concourse.bacc as bacc
import concourse.bass as bass
import concourse.mybir as mybir
import concourse.tile as tile
from concourse.bass_utils import run_bass_kernel_spmd
from concourse.masks import make_identity

P = 128
N = 100000
E = 1600000
M = 8
NQ = 4                       # src chunks (int16 index ranges)
NPC = N // M                 # 12500
BPC = (NPC + P - 1) // P     # 98
SLICE = BPC * P              # 12544
CHN = N // NQ                # 25000 nodes per chunk
CIN = 128
CH = 128                     # unified on-device column count (incl layer 3)
COUT = 64
EPC = E // M                 # 200000

SB = 8         # S-build batch (tiles per DVE op)
GBT = 40       # target gather batch (tiles per dma_gather call)
BDD = 32       # decode tiles per gather batch

PHASES = None  # None = all; else set of phase names to include


def _configure(n, e):
    """Switch module to a smaller problem size (for fast validation)."""
    global N, E, NPC, BPC, SLICE, EPC, CHN
    N, E = n, e
    NPC = N // M
    BPC = (NPC + P - 1) // P
    SLICE = BPC * P
    EPC = E // M
    CHN = N // NQ

f32 = mybir.dt.float32
f16 = mybir.dt.float16
i16 = mybir.dt.int16
i32 = mybir.dt.int32


def _wrap16(idx):
    """[n] int array -> dma_gather idx table [128, n//16] int16
    (idx i at [i%16, i//16], replicated to all 8 16-partition groups)."""
    n = idx.size
    a = np.ascontiguousarray(idx.reshape(n // 16, 16).T.astype(np.int16))
    return np.ascontiguousarray(np.tile(a, (8, 1)))


# --------------------------- host preprocessing ---------------------------

def _prep_agg(pos_edge_index):
    # deg/dinv INCLUDE self-loops; the edge stream EXCLUDES them (the
    # self-loop term is added in the aggregation epilogue from hhat).
    src = pos_edge_index[0]
    dst = pos_edge_index[1]
    deg = (np.bincount(dst, minlength=N) + 1).astype(np.float32)
    dinv = (1.0 / np.sqrt(deg)).astype(np.float32)

    q_all = src // CHN
    order = np.lexsort((q_all, dst))
    src_s, dst_s, q_s = src[order], dst[order], q_all[order]
    core_of = dst_s // NPC

    # per-(core, block, q) counts -> uniform K_q
    blk_all = (dst_s - core_of * NPC) // P
    counts = np.zeros((M, BPC, NQ), dtype=np.int64)
    np.add.at(counts, (core_of, blk_all, q_s), 1)
    Kq = [int(np.max((counts[:, :, q] + P - 1) // P)) for q in range(NQ)]
    Tt = BPC * int(np.sum(Kq))
    qbase = np.concatenate([[0], np.cumsum(Kq)]) * BPC  # tile base per pass

    cores = []
    for c in range(M):
        sel = core_of == c
        s_c, d_c, q_c = src_s[sel], dst_s[sel], q_s[sel]
        b_c = (d_c - c * NPC) // P
        key = b_c * NQ + q_c
        ord2 = np.argsort(key, kind="stable")
        s_c, d_c, key = s_c[ord2], d_c[ord2], key[ord2]
        srcL = np.zeros(Tt * P, dtype=np.int64)
        dstloc = np.full(Tt * P, -1.0, dtype=np.float32)
        bounds = np.searchsorted(key, np.arange(BPC * NQ + 1))
        for b in range(BPC):
            for q in range(NQ):
                lo, hi = bounds[b * NQ + q], bounds[b * NQ + q + 1]
                n_e = hi - lo
                base = (qbase[q] + b * Kq[q]) * P
                srcL[base:base + n_e] = s_c[lo:hi] - q * CHN
                dstloc[base:base + n_e] = (d_c[lo:hi] -
                                           (c * NPC + b * P)).astype(np.float32)
        dinvT = np.zeros((P, BPC), dtype=np.float32)
        for b in range(BPC):
            lo = c * NPC + b * P
            hi = min(lo + P, (c + 1) * NPC)
            dinvT[:hi - lo, b] = dinv[lo:hi]
        cores.append(dict(
            srcT16=_wrap16(srcL),
            dstlocT=np.ascontiguousarray(
                dstloc.reshape(Tt, P).T.astype(np.float16)),
            dinvT=dinvT))
    return Kq, Tt, cores


def _prep_decode(pe, ne):
    """Sort each core's edges by (src chunk, dst chunk); uniform TG tiles
    per group. Returns (TG, per-set per-core dict with idx tables + pos)."""
    sets = [pe, ne]
    keys, counts = [], np.zeros((2, M, NQ * NQ), np.int64)
    for si, ei in enumerate(sets):
        kk = []
        for c in range(M):
            s = ei[0, c * EPC:(c + 1) * EPC]
            d = ei[1, c * EPC:(c + 1) * EPC]
            key = (s // CHN) * NQ + (d // CHN)
            np.add.at(counts[si, c], key, 1)
            kk.append(key)
        keys.append(kk)
    TG = int(np.max((counts + P - 1) // P))
    NT = NQ * NQ * TG
    out = []
    for si, ei in enumerate(sets):
        cores = []
        for c in range(M):
            s = ei[0, c * EPC:(c + 1) * EPC].astype(np.int64)
            d = ei[1, c * EPC:(c + 1) * EPC].astype(np.int64)
            key = keys[si][c]
            order = np.argsort(key, kind="stable")
            s_o, d_o, key_o = s[order], d[order], key[order]
            sL = np.zeros(NT * P, dtype=np.int64)
            dL = np.zeros(NT * P, dtype=np.int64)
            pos = np.full(NT * P, -1, dtype=np.int64)
            bounds = np.searchsorted(key_o, np.arange(NQ * NQ + 1))
            for g in range(NQ * NQ):
                lo, hi = bounds[g], bounds[g + 1]
                n_e = hi - lo
                base = g * TG * P
                sL[base:base + n_e] = s_o[lo:hi] - (g // NQ) * CHN
                dL[base:base + n_e] = d_o[lo:hi] - (g % NQ) * CHN
                pos[base:base + n_e] = order[lo:hi]
            cores.append(dict(sidxT=_wrap16(sL), didxT=_wrap16(dL), pos=pos))
        out.append(cores)
    return TG, out


# ----------------------------- device builder -----------------------------

def build_nc(Kq, TG, reps=1):
    Tt = BPC * int(np.sum(Kq))
    qbase = np.concatenate([[0], np.cumsum(Kq)]) * BPC
    NT = NQ * NQ * TG
    DGd = (NT + P - 1) // P          # decode transpose groups
    NTP = DGd * P
    nc = bacc.Bacc(None, target_bir_lowering=False)
    with tile.TileContext(nc) as tc:
        with tc.tile_pool(name="dram", bufs=1, space="DRAM") as dram, \
             tc.tile_pool(name="cst", bufs=1) as cst, \
             tc.tile_pool(name="xt", bufs=4) as xtp, \
             tc.tile_pool(name="sS", bufs=4) as sSp, \
             tc.tile_pool(name="msg", bufs=3) as msgp, \
             tc.tile_pool(name="prod", bufs=2) as prodp, \
             tc.tile_pool(name="zb", bufs=4) as zbp, \
             tc.tile_pool(name="ps", bufs=2, space="PSUM") as psp, \
             tc.tile_pool(name="acc", bufs=2, space="PSUM") as accp:

            # ---------------- I/O ----------------
            def ein(name, shape, dtype=f32):
                return dram.tile(shape, dtype, kind="ExternalInput",
                                 name=name, uniquify=False)

            x_s = ein("x_s", [SLICE, CIN], f16)
            W1 = ein("W1", [CIN, CH], f16); W2 = ein("W2", [CH, CH], f16)
            W3 = ein("W3", [CH, CH], f16)          # host-padded to 128 cols
            bb1 = ein("bb1", [P, CH]); bb2 = ein("bb2", [P, CH])
            bb3 = ein("bb3", [P, CH])              # host-padded
            dinvT = ein("dinvT", [P, BPC])
            srcT16 = ein("srcT16", [P, Tt * 8], i16)
            dstlocT = ein("dstlocT", [P, Tt], f16)
            ps_idx = ein("ps_idx", [P, NT * 8], i16)
            pd_idx = ein("pd_idx", [P, NT * 8], i16)
            ns_idx = ein("ns_idx", [P, NT * 8], i16)
            nd_idx = ein("nd_idx", [P, NT * 8], i16)

            pos_out = dram.tile([NTP, P], f32, kind="ExternalOutput",
                                name="pos_out", uniquify=False)
            neg_out = dram.tile([NTP, P], f32, kind="ExternalOutput",
                                name="neg_out", uniquify=False)

            # internal DRAM
            hs1 = dram.tile([SLICE, CH], f16, name="hs1")
            hs2 = dram.tile([SLICE, CH], f16, name="hs2")
            hs3 = dram.tile([SLICE, CH], f16, name="hs3")
            z3s = dram.tile([SLICE, CH], f16, name="z3s")

            # ---------------- constants to SBUF ----------------
            W1_sb = cst.tile([CIN, CH], f16)
            W2_sb = cst.tile([CH, CH], f16)
            W3_sb = cst.tile([CH, CH], f16)
            bb1_sb = cst.tile([P, CH], f32)
            bb2_sb = cst.tile([P, CH], f32)
            bb3_sb = cst.tile([P, CH], f32)
            dinv_sb = cst.tile([P, BPC], f32)
            dstloc_sb = cst.tile([P, Tt], f16)
            for dst_t, src_t in [(W1_sb, W1), (W2_sb, W2), (W3_sb, W3),
                                 (bb1_sb, bb1), (bb2_sb, bb2), (bb3_sb, bb3),
                                 (dinv_sb, dinvT), (dstloc_sb, dstlocT)]:
                nc.sync.dma_start(out=dst_t[:], in_=src_t[:])

            ident = cst.tile([P, P], f32)
            make_identity(nc, ident[:])
            ident_h = cst.tile([P, P], f16)
            nc.vector.tensor_copy(out=ident_h[:], in_=ident[:])
            iota_i = cst.tile([P, P], i32)
            nc.gpsimd.iota(iota_i[:], pattern=[[1, P]], base=0,
                           channel_multiplier=0)
            iota_f = cst.tile([P, P], f16)
            nc.vector.tensor_copy(out=iota_f[:], in_=iota_i[:])

            # persistent SBUF state
            z_sb = cst.tile([P, BPC * CH], f16)        # z1 / z2 slice
            acc_sb = cst.tile([P, BPC * CH], f32)      # agg accumulator
            score_sb = cst.tile([P, NTP], f32)
            nc.vector.memset(score_sb[:], 0.0)

            # ---------------- phases ----------------
            def dense(layer, W_sb, hs_out, scope):
                with nc.named_scope(scope):
                    for b in range(BPC):
                        if layer == 1:
                            zt = xtp.tile([P, CIN], f16, tag="zt")
                            nc.sync.dma_start(out=zt[:],
                                              in_=x_s[b * P:(b + 1) * P, :])
                            src_ap = zt[:]
                        else:
                            src_ap = z_sb[:, b * CH:(b + 1) * CH]
                        tp = psp.tile([P, CH], f16, tag="tp")
                        nc.tensor.transpose(out=tp[:], in_=src_ap,
                                            identity=ident_h[:])
                        zT = xtp.tile([P, CH], f16, tag="zT")
                        nc.vector.tensor_copy(out=zT[:], in_=tp[:])
                        hp = psp.tile([P, CH], f32, tag="hp")
                        nc.tensor.matmul(out=hp[:], lhsT=zT[:], rhs=W_sb[:],
                                         start=True, stop=True)
                        hh = zbp.tile([P, CH], f16, tag="hh")
                        nc.vector.tensor_scalar(
                            out=hh[:], in0=hp[:],
                            scalar1=dinv_sb[:, b:b + 1], scalar2=None,
                            op0=mybir.AluOpType.mult)
                        nc.sync.dma_start(
                            out=hs_out[b * P:(b + 1) * P, :], in_=hh[:])

            def allgather(slice_t, full_t, scope):
                with nc.named_scope(scope):
                    nc.gpsimd.collective_compute(
                        "AllGather", mybir.AluOpType.bypass,
                        replica_groups=[list(range(M))],
                        ins=[slice_t[:NPC, :]],
                        outs=[full_t[:]])

            def agg(hf, hs_in, bias_sb, relu, scope):
                with nc.named_scope(scope):
                    for q in range(NQ):
                        KQ = Kq[q]
                        gbb = max(1, GBT // KQ)      # blocks per gather call
                        for b0 in range(0, BPC, gbb):
                            nblk = min(gbb, BPC - b0)
                            ntile = nblk * KQ
                            t0 = int(qbase[q]) + b0 * KQ
                            aidx = xtp.tile([P, gbb * KQ * 8], i16,
                                            tag="aidx")
                            nc.sync.dma_start(
                                out=aidx[:, :ntile * 8],
                                in_=srcT16[:, t0 * 8:(t0 + ntile) * 8])
                            msg = msgp.tile([P, gbb * KQ, CH], f16, tag="msg")
                            nc.gpsimd.dma_gather(
                                msg[:, :ntile, :],
                                hf[q * CHN:(q + 1) * CHN, :],
                                aidx[:, :ntile * 8],
                                ntile * P, ntile * P, CH,
                                single_packet=False)
                            for bi in range(nblk):
                                b = b0 + bi
                                seg = accp.tile([P, CH], f32, tag="seg")
                                for k0 in range(0, KQ, SB):
                                    nb = min(SB, KQ - k0)
                                    S4 = sSp.tile([P, SB, P], f16, tag="S4")
                                    tt = t0 + bi * KQ + k0
                                    nc.vector.tensor_tensor(
                                        out=S4[:, :nb, :],
                                        in0=dstloc_sb[:, tt:tt + nb]
                                            .unsqueeze(2)
                                            .to_broadcast([P, nb, P]),
                                        in1=iota_f[:].unsqueeze(1)
                                            .to_broadcast([P, nb, P]),
                                        op=mybir.AluOpType.is_equal)
                                    for j in range(nb):
                                        k = k0 + j
                                        nc.tensor.matmul(
                                            out=seg[:], lhsT=S4[:, j, :],
                                            rhs=msg[:, bi * KQ + k, :],
                                            start=(k == 0),
                                            stop=(k == KQ - 1))
                                a_sl = acc_sb[:, b * CH:(b + 1) * CH]
                                if q == 0:
                                    nc.vector.tensor_copy(out=a_sl, in_=seg[:])
                                else:
                                    nc.vector.tensor_tensor(
                                        out=a_sl, in0=a_sl, in1=seg[:],
                                        op=mybir.AluOpType.add)
                    for b in range(BPC):
                        # self-loop: z = (acc + hhat_own)*dinv + bias
                        hho = zbp.tile([P, CH], f16, tag="hho")
                        nc.sync.dma_start(
                            out=hho[:], in_=hs_in[b * P:(b + 1) * P, :])
                        t1 = zbp.tile([P, CH], f32, tag="t1")
                        nc.vector.tensor_scalar(
                            out=t1[:], in0=hho[:],
                            scalar1=dinv_sb[:, b:b + 1], scalar2=None,
                            op0=mybir.AluOpType.mult)
                        zb = zbp.tile([P, CH], f32, tag="zbE")
                        nc.vector.tensor_scalar(
                            out=zb[:], in0=acc_sb[:, b * CH:(b + 1) * CH],
                            scalar1=dinv_sb[:, b:b + 1], scalar2=None,
                            op0=mybir.AluOpType.mult)
                        nc.vector.tensor_tensor(
                            out=zb[:], in0=zb[:], in1=t1[:],
                            op=mybir.AluOpType.add)
                        if relu:
                            nc.vector.tensor_tensor(
                                out=zb[:], in0=zb[:], in1=bias_sb[:],
                                op=mybir.AluOpType.add)
                            nc.vector.tensor_scalar_max(
                                z_sb[:, b * CH:(b + 1) * CH], zb[:], 0.0)
                        else:
                            zb3 = zbp.tile([P, CH], f16, tag="zb3")
                            nc.vector.tensor_tensor(
                                out=zb3[:], in0=zb[:], in1=bias_sb[:],
                                op=mybir.AluOpType.add)
                            nc.sync.dma_start(
                                out=z3s[b * P:(b + 1) * P, :], in_=zb3[:])

            def decode(z3f, sidx_d, didx_d, out_t, scope):
                with nc.named_scope(scope):
                    for g in range(NQ * NQ):
                        qs, qd = g // NQ, g % NQ
                        for off in range(0, TG, BDD):
                            nt = min(BDD, TG - off)
                            t0 = g * TG + off
                            sidx = xtp.tile([P, BDD * 8], i16, tag="sidx")
                            didx = xtp.tile([P, BDD * 8], i16, tag="didx")
                            nc.sync.dma_start(
                                out=sidx[:, :nt * 8],
                                in_=sidx_d[:, t0 * 8:(t0 + nt) * 8])
                            nc.sync.dma_start(
                                out=didx[:, :nt * 8],
                                in_=didx_d[:, t0 * 8:(t0 + nt) * 8])
                            za = msgp.tile([P, BDD, CH], f16, tag="za")
                            zbt = msgp.tile([P, BDD, CH], f16, tag="zbt")
                            nc.gpsimd.dma_gather(
                                za[:, :nt, :],
                                z3f[qs * CHN:(qs + 1) * CHN, :],
                                sidx[:, :nt * 8], nt * P, nt * P, CH,
                                single_packet=False)
                            nc.gpsimd.dma_gather(
                                zbt[:, :nt, :],
                                z3f[qd * CHN:(qd + 1) * CHN, :],
                                didx[:, :nt * 8], nt * P, nt * P, CH,
                                single_packet=False)
                            prod = prodp.tile([P, BDD, CH], f16, tag="prod")
                            nc.vector.tensor_tensor(
                                out=prod[:, :nt, :], in0=za[:, :nt, :],
                                in1=zbt[:, :nt, :], op=mybir.AluOpType.mult)
                            nc.vector.tensor_reduce(
                                out=score_sb[:, t0:t0 + nt].unsqueeze(2),
                                in_=prod[:, :nt, :],
                                axis=mybir.AxisListType.X,
                                op=mybir.AluOpType.add)
                    for gg in range(DGd):
                        tp = psp.tile([P, P], f32, tag="tpS")
                        nc.tensor.transpose(
                            out=tp[:], in_=score_sb[:, gg * P:(gg + 1) * P],
                            identity=ident[:])
                        so = zbp.tile([P, P], f32, tag="so")
                        nc.vector.tensor_copy(out=so[:], in_=tp[:])
                        nc.sync.dma_start(
                            out=out_t[gg * P:(gg + 1) * P, :], in_=so[:])

            def on(p):
                return PHASES is None or p in PHASES

            def run_pipeline(r):
                # Shared (collective-output) DRAM tensors allow only a
                # single writer instruction -> one set per rep.
                hf1 = dram.tile([N, CH], f16, name=f"hf1_r{r}",
                                addr_space="Shared")
                hf2 = dram.tile([N, CH], f16, name=f"hf2_r{r}",
                                addr_space="Shared")
                hf3 = dram.tile([N, CH], f16, name=f"hf3_r{r}",
                                addr_space="Shared")
                z3f = dram.tile([N, CH], f16, name=f"z3f_r{r}",
                                addr_space="Shared")
                if on("dense1"):
                    dense(1, W1_sb, hs1, "dense1")
                if on("ag1"):
                    allgather(hs1, hf1, "ag1")
                if on("agg1"):
                    agg(hf1, hs1, bb1_sb, True, "agg1")
                if on("dense2"):
                    dense(2, W2_sb, hs2, "dense2")
                if on("ag2"):
                    allgather(hs2, hf2, "ag2")
                if on("agg2"):
                    agg(hf2, hs2, bb2_sb, True, "agg2")
                if on("dense3"):
                    dense(3, W3_sb, hs3, "dense3")
                if on("ag3"):
                    allgather(hs3, hf3, "ag3")
                if on("agg3"):
                    agg(hf3, hs3, bb3_sb, False, "agg3")
                if on("ag4"):
                    allgather(z3s, z3f, "ag4")
                if on("dec"):
                    decode(z3f, ps_idx, pd_idx, pos_out, "dec_pos")
                    decode(z3f, ns_idx, nd_idx, neg_out, "dec_neg")

            for r in range(reps):
                run_pipeline(r)
    nc.compile()
    return nc


_CACHE = {}


def _make_in_maps(x, W1, b1, W2, b2, W3, b3, pe, ne):
    Kq, Tt, agg_cores = _prep_agg(pe)
    TG, dec = _prep_decode(pe, ne)
    W3p = np.zeros((CH, CH), np.float16)
    W3p[:, :COUT] = W3.astype(np.float16)
    b3p = np.zeros(CH, np.float32)
    b3p[:COUT] = b3
    in_maps = []
    pos_maps = []
    for c in range(M):
        xs = np.zeros((SLICE, CIN), np.float16)
        xs[:NPC] = x[c * NPC:(c + 1) * NPC].astype(np.float16)
        a = agg_cores[c]
        in_maps.append({
            "x_s": xs,
            "W1": W1.astype(np.float16), "W2": W2.astype(np.float16),
            "W3": W3p,
            "bb1": np.tile(b1[None, :], (P, 1)).astype(np.float32),
            "bb2": np.tile(b2[None, :], (P, 1)).astype(np.float32),
            "bb3": np.tile(b3p[None, :], (P, 1)).astype(np.float32),
            "dinvT": a["dinvT"], "srcT16": a["srcT16"],
            "dstlocT": a["dstlocT"],
            "ps_idx": dec[0][c]["sidxT"], "pd_idx": dec[0][c]["didxT"],
            "ns_idx": dec[1][c]["sidxT"], "nd_idx": dec[1][c]["didxT"],
        })
        pos_maps.append((dec[0][c]["pos"], dec[1][c]["pos"]))
    return (Kq, TG), (in_maps, pos_maps)


def _run(run_args, key_args, reps=1):
    in_maps, pos_maps = run_args
    Kq, TG = key_args
    key = (tuple(Kq), TG, reps)
    if key not in _CACHE:
        _CACHE[key] = build_nc(list(Kq), TG, reps=reps)
    res = run_bass_kernel_spmd(_CACHE[key], in_maps,
                               core_ids=list(range(M)))
    pos = np.empty(E, np.float32)
    neg = np.empty(E, np.float32)
    for c in range(M):
        for name, pos_arr, out in [("pos_out", pos_maps[c][0], pos),
                                   ("neg_out", pos_maps[c][1], neg)]:
            flat = res.results[c][name].ravel()
            valid = pos_arr >= 0
            out[c * EPC:(c + 1) * EPC][pos_arr[valid]] = \
                flat[:pos_arr.size][valid]
    return pos, neg


def kernel(x, W1, b1, W2, b2, W3, b3, pos_edge_index, neg_edge_index):
    x = np.asarray(x, dtype=np.float32)
    W1 = np.asarray(W1, np.float32); b1 = np.asarray(b1, np.float32)
    W2 = np.asarray(W2, np.float32); b2 = np.asarray(b2, np.float32)
    W3 = np.asarray(W3, np.float32); b3 = np.asarray(b3, np.float32)
    pe = np.asarray(pos_edge_index).astype(np.int64)
    ne = np.asarray(neg_edge_index).astype(np.int64)
    key_args, run_args = _make_in_maps(x, W1, b1, W2, b2, W3, b3, pe, ne)
    return _run(run_args, key_args, reps=1)
